# revision 1
# baseline (speedup 1.0000x reference)
"""4-layer GCN encoder on 8 Trainium2 NeuronCores.

Strategy (graph/data parallel, dst-node sharding):
  - Nodes are permuted into 8*NB blocks of 128 (balanced by in-degree) and
    sharded across 8 cores by destination.
  - Layer 1 dense projection x@W1 is computed fully on every core (cheaper
    than an AllGather of the result); layers 2-4 compute only the local node
    shard and AllGather the projected features.
  - Aggregation: per-edge gather of source rows via the SWDGE dma_gather
    instruction (int16 indices -> source-half split), then scatter-add via
    TensorEngine matmuls against on-chip-generated one-hot matrices S with
    the per-edge GCN norm as values (PSUM accumulation per dst block).
  - All matmul operands bf16, accumulation fp32.
"""

import math
import numpy as np
import ml_dtypes

import concourse.bacc as bacc
import concourse.mybir as mybir
import concourse.tile as tile
from concourse.bass_utils import run_bass_kernel_spmd

P = 128
BF16 = mybir.dt.bfloat16
F32 = mybir.dt.float32
I16 = mybir.dt.int16


class Cfg:
    def __init__(self, n_nodes=50000, n_edges=800000, in_ch=512, hid=256,
                 ncores=8, nb=49, G=7):
        self.n_nodes = n_nodes
        self.n_edges = n_edges
        self.in_ch = in_ch
        self.hid = hid
        self.ncores = ncores
        self.nb = nb                      # dst blocks of 128 per core
        self.G = G                        # blocks per gather group
        assert nb % G == 0
        self.NG = nb // G                 # groups per core
        self.shard = nb * P               # nodes per core (padded)
        self.npad = ncores * self.shard   # padded total nodes
        assert self.npad >= n_nodes
        self.half = self.npad // 2        # src-half boundary for int16 idx
        assert self.half % P == 0 and self.half < 32768
        self.fc_in = in_ch // P           # K chunks for layer 1
        self.fh = hid // P                # feature halves (2)
        assert self.fh == 2
        self.slab = 8                     # n-tiles per L1 x slab


CFG = Cfg()


# ----------------------------------------------------------------- host prep

def _preprocess(cfg, edge_index, edge_weight):
    """Numpy preprocessing: norms, balanced node permutation, per-core
    padded edge structures. Returns dict."""
    N = cfg.n_nodes
    src0 = np.asarray(edge_index[0], dtype=np.int64)
    dst0 = np.asarray(edge_index[1], dtype=np.int64)
    ew0 = np.asarray(edge_weight, dtype=np.float32)
    # self loops (PyG gcn_norm, fill=1)
    loops = np.arange(N, dtype=np.int64)
    src = np.concatenate([src0, loops])
    dst = np.concatenate([dst0, loops])
    ew = np.concatenate([ew0, np.ones(N, np.float32)])
    deg = np.bincount(dst, weights=ew.astype(np.float64), minlength=N)
    deg = deg.astype(np.float32)
    dis = np.where(deg > 0, 1.0 / np.sqrt(np.where(deg > 0, deg, 1.0)), 0.0)
    dis = dis.astype(np.float32)
    norm = dis[src] * ew * dis[dst]

    # balanced block assignment: round-robin of degree-sorted nodes
    NBT = cfg.ncores * cfg.nb
    degc = np.bincount(dst, minlength=N)          # in-edge counts per node
    order = np.argsort(-degc, kind="stable")
    blk_of_rank = np.arange(N) % NBT
    pos_of_rank = np.arange(N) // NBT
    assert pos_of_rank.max() < P, "block capacity exceeded"
    gslot = np.empty(N, dtype=np.int64)
    gslot[order] = blk_of_rank * P + pos_of_rank

    ps = gslot[src]
    pd = gslot[dst]

    # per (core, block, half) edge counts to find uniform tile count T
    eb = pd // P                                  # global dst block per edge
    ehalf = (ps >= cfg.half).astype(np.int64)
    key = eb * 2 + ehalf
    cnt = np.bincount(key, minlength=NBT * 2)
    T = max(1, int(np.ceil(cnt.max() / P)))

    nslots = cfg.nb * 2 * T                       # tiles per core
    cap = nslots * P
    gidx16 = np.zeros((cfg.ncores, 16, cap // 16), dtype=np.int16)
    dstc = np.zeros((cfg.ncores, P, nslots), dtype=np.float32)
    normc = np.zeros((cfg.ncores, P, nslots), dtype=np.float32)

    # global ordering of edges: core -> (group, half, block-in-group, tile)
    core_e = eb // cfg.nb
    b_in_core = eb % cfg.nb
    g = b_in_core // cfg.G
    bg = b_in_core % cfg.G
    # slot (tile) base for each edge's (b, h) bucket
    srt = np.lexsort((ps, ehalf, eb))             # sort edges by (block, half, src)
    # rank within bucket
    key_s = key[srt]
    # compute rank-in-bucket via cumcount
    uniq, inv, counts = np.unique(key_s, return_inverse=True, return_counts=True)
    starts = np.zeros_like(counts)
    starts[1:] = np.cumsum(counts)[:-1]
    rank_in_bucket = np.arange(len(srt)) - starts[inv]

    es = srt                                      # edge order
    t_idx = rank_in_bucket // P                   # tile within bucket
    j_idx = rank_in_bucket % P                    # lane within tile
    assert t_idx.max() < T
    sg = g[es]
    sh = ehalf[es]
    sbg = bg[es]
    s_slot = ((sg * 2 + sh) * cfg.G + sbg) * T + t_idx
    q = s_slot * P + j_idx                        # position within core arrays
    score = core_e[es]
    idxval = np.where(sh == 1, ps[es] - cfg.half, ps[es]).astype(np.int16)
    dlocal = (pd[es] % P).astype(np.float32)
    nval = norm[es]

    for c in range(cfg.ncores):
        m = score == c
        qc = q[m]
        gidx16[c, qc % 16, qc // 16] = idxval[m]
        dstc[c, qc % P, qc // P] = dlocal[m]
        normc[c, qc % P, qc // P] = nval[m]

    gidx = np.tile(gidx16, (1, 8, 1))             # replicate to 128 partitions
    inv_gslot = gslot                             # y[v] = yperm[gslot[v]]
    return dict(T=T, nslots=nslots, gidx=gidx,
                dstc=dstc.astype(ml_dtypes.bfloat16),
                normc=normc.astype(ml_dtypes.bfloat16),
                gslot=inv_gslot)


def _pack_xts(cfg, x, gslot):
    """Host: permuted, transposed, slab-tiled x for layer-1 lhsT streaming.
    Layout [fc, s, p, t*128+c] = x_perm[(s*8+t)*128+c, fc*128+p]."""
    xpad = np.zeros((cfg.npad, cfg.in_ch), dtype=np.float32)
    xpad[gslot] = x
    nslab = cfg.npad // (cfg.slab * P)
    a = xpad.T.reshape(cfg.fc_in, P, nslab, cfg.slab, P)
    a = a.transpose(0, 2, 1, 3, 4).reshape(cfg.fc_in, nslab, P, cfg.slab * P)
    return np.ascontiguousarray(a.astype(ml_dtypes.bfloat16)).reshape(
        cfg.fc_in * nslab * P, cfg.slab * P)


def _pack_wcat(cfg, Ws):
    """[128, (fc_in + 3*fh)*hid] bf16 : W1 chunks then W2..W4 chunks."""
    cols = []
    for Wl in Ws:
        k = Wl.shape[0]
        for fc in range(k // P):
            cols.append(Wl[fc * P:(fc + 1) * P, :])
    return np.concatenate(cols, axis=1).astype(ml_dtypes.bfloat16)


def _pack_bias(cfg, bs):
    out = np.zeros((P, 2 * len(bs)), dtype=np.float32)
    for l, b in enumerate(bs):
        for fh in range(cfg.fh):
            out[:, l * 2 + fh] = b[fh * P:(fh + 1) * P]
    return out


def _iota_np():
    return np.tile(np.arange(P, dtype=np.float32)[None, :], (P, 1)).astype(
        ml_dtypes.bfloat16)


# ----------------------------------------------------------------- builder

def _build(cfg, T, n_layers=4, debug_dense=False):
    nslots = cfg.nb * 2 * T
    HID = cfg.hid
    nc = bacc.Bacc("TRN2", target_bir_lowering=False, debug=False,
                   num_devices=cfg.ncores, num_swdge_queues=4)
    qctr = [0]

    gidx_d = nc.dram_tensor("gidx", [P, nslots * 8], I16, kind="ExternalInput")
    dstc_d = nc.dram_tensor("dstc", [P, nslots], BF16, kind="ExternalInput")
    normc_d = nc.dram_tensor("normc", [P, nslots], BF16, kind="ExternalInput")
    iota_d = nc.dram_tensor("iota", [P, P], BF16, kind="ExternalInput")
    wcat_cols = (cfg.fc_in + 3 * cfg.fh) * HID
    wcat_d = nc.dram_tensor("wcat", [P, wcat_cols], BF16, kind="ExternalInput")
    bias_d = nc.dram_tensor("bias", [P, 8], F32, kind="ExternalInput")
    prelu_d = nc.dram_tensor("prelua", [P, 2], F32, kind="ExternalInput")
    nslab = cfg.npad // (cfg.slab * P)
    xts_d = nc.dram_tensor("xts", [cfg.fc_in * nslab * P, cfg.slab * P], BF16,
                           kind="ExternalInput")
    out_d = nc.dram_tensor("out", [cfg.fh * cfg.nb * P, P], F32,
                           kind="ExternalOutput")

    w_off = {}
    off = 0
    for l in range(4):
        k = cfg.fc_in if l == 0 else cfg.fh
        for fc in range(k):
            w_off[(l, fc)] = off
            off += HID

    with tile.TileContext(nc) as tc:
        with (
            tc.tile_pool(name="res", bufs=1) as res,
            tc.tile_pool(name="mpool", bufs=2) as mpool,
            tc.tile_pool(name="spool", bufs=2) as spool,
            tc.tile_pool(name="xpool", bufs=2) as xpool,
            tc.tile_pool(name="apool", bufs=4) as apool,
            tc.tile_pool(name="htpool", bufs=1) as htpool,
            tc.tile_pool(name="opool", bufs=4) as opool,
            tc.tile_pool(name="ppool", bufs=cfg.G, space="PSUM") as ppool,
            tc.tile_pool(name="dpsum", bufs=1, space="PSUM") as dpsum,
            tc.tile_pool(name="dram", bufs=2, space="DRAM") as dram,
        ):
            # ---- resident loads
            gidx = res.tile([P, nslots * 8], I16)
            nc.sync.dma_start(out=gidx[:], in_=gidx_d[:])
            dstc = res.tile([P, nslots], BF16)
            nc.sync.dma_start(out=dstc[:], in_=dstc_d[:])
            normc = res.tile([P, nslots], BF16)
            nc.sync.dma_start(out=normc[:], in_=normc_d[:])
            iota = res.tile([P, P], BF16)
            nc.sync.dma_start(out=iota[:], in_=iota_d[:])
            wcat = res.tile([P, wcat_cols], BF16)
            nc.sync.dma_start(out=wcat[:], in_=wcat_d[:])
            bias = res.tile([P, 8], F32)
            nc.sync.dma_start(out=bias[:], in_=bias_d[:])
            prelua = res.tile([P, 2], F32)
            nc.sync.dma_start(out=prelua[:], in_=prelu_d[:])

            hT = {}

            def dense_full_l1(a_hA, a_hB):
                htiles = cfg.half // P
                for s in range(nslab):
                    xsl = [xpool.tile([P, cfg.slab * P], BF16, tag=f"x{fc}", name=f"xsl{fc}")
                           for fc in range(cfg.fc_in)]
                    for fc in range(cfg.fc_in):
                        base = (fc * nslab + s) * P
                        nc.sync.dma_start(out=xsl[fc][:],
                                          in_=xts_d[base:base + P, :])
                    for t in range(cfg.slab):
                        nt = s * cfg.slab + t
                        pd_ = dpsum.tile([P, HID], F32, tag="dps", name="pd1")
                        for fc in range(cfg.fc_in):
                            nc.tensor.matmul(
                                out=pd_[:],
                                lhsT=xsl[fc][:, t * P:(t + 1) * P],
                                rhs=wcat[:, w_off[(0, fc)]:w_off[(0, fc)] + HID],
                                start=(fc == 0), stop=(fc == cfg.fc_in - 1))
                        asb = apool.tile([P, HID], BF16, tag="asb", name="asb1")
                        nc.scalar.copy(out=asb[:], in_=pd_[:])
                        if nt < htiles:
                            dst = a_hA[nt * P:(nt + 1) * P, :]
                        else:
                            dst = a_hB[(nt - htiles) * P:(nt - htiles + 1) * P, :]
                        nc.sync.dma_start(out=dst, in_=asb[:])

            def dense_shard(l, a_shard):
                for nt in range(cfg.nb):
                    pd_ = dpsum.tile([P, HID], F32, tag="dps", name="pd2")
                    for fc in range(cfg.fh):
                        nc.tensor.matmul(
                            out=pd_[:],
                            lhsT=hT[(fc, nt)][:],
                            rhs=wcat[:, w_off[(l, fc)]:w_off[(l, fc)] + HID],
                            start=(fc == 0), stop=(fc == cfg.fh - 1))
                    asb = apool.tile([P, HID], BF16, tag="asb", name="asb2")
                    nc.scalar.copy(out=asb[:], in_=pd_[:])
                    nc.sync.dma_start(
                        out=a_shard[nt * P:(nt + 1) * P, :], in_=asb[:])

            def aggregate(l, src_aps):
                for g in range(cfg.NG):
                    pb = {}
                    for h in range(2):
                        call_off = (g * 2 + h) * cfg.G * T * 8
                        M = mpool.tile([P, cfg.G * T * HID], BF16, tag="M", name="M")
                        src_ap = src_aps[h]
                        CT = 8          # tiles per gather call (<=1024 idx)
                        for k0 in range(0, cfg.G * T, CT):
                            k1 = min(k0 + CT, cfg.G * T)
                            nt_ = k1 - k0
                            nc.gpsimd.dma_gather(
                                out_ap=M[:, k0 * HID:k1 * HID].rearrange(
                                    "p (t e) -> p t e", e=HID),
                                in_ap=src_ap,
                                idxs_ap=gidx[:, call_off + k0 * 8:
                                             call_off + k1 * 8],
                                num_idxs=nt_ * P,
                                num_idxs_reg=nt_ * P,
                                elem_size=HID,
                                queue_num=qctr[0] % 4,
                            )
                            qctr[0] += 1
                        S = spool.tile([P, cfg.G * T * P], BF16, tag="S", name="S")
                        for bg in range(cfg.G):
                            slot0 = ((g * 2 + h) * cfg.G + bg) * T
                            s3 = S[:, bg * T * P:(bg + 1) * T * P].rearrange(
                                "p (t e) -> p t e", e=P)
                            iob = iota[:].rearrange(
                                "p (o e) -> p o e", o=1).broadcast_to([P, T, P])
                            nc.vector.tensor_tensor(
                                out=s3, in0=iob,
                                in1=dstc[:, slot0:slot0 + T].to_broadcast([P, T, P]),
                                op=mybir.AluOpType.is_equal)
                            nc.vector.tensor_tensor(
                                out=s3, in0=s3,
                                in1=normc[:, slot0:slot0 + T].to_broadcast([P, T, P]),
                                op=mybir.AluOpType.mult)
                        for bg in range(cfg.G):
                            if h == 0:
                                pb[bg] = ppool.tile([P, HID], F32, tag="pb", name=f"pb")
                            for t in range(T):
                                tl = bg * T + t
                                for fh in range(cfg.fh):
                                    nc.tensor.matmul(
                                        out=pb[bg][:, fh * P:(fh + 1) * P],
                                        lhsT=M[:, tl * HID + fh * P:
                                               tl * HID + (fh + 1) * P],
                                        rhs=S[:, tl * P:(tl + 1) * P],
                                        start=(h == 0 and t == 0 and fh == 0),
                                        stop=(h == 1 and t == T - 1 and fh == 1))
                    # epilogue for the group's blocks
                    for bg in range(cfg.G):
                        nt = g * cfg.G + bg
                        for fh in range(cfg.fh):
                            pslice = pb[bg][:, fh * P:(fh + 1) * P]
                            bcol = bias[:, l * 2 + fh:l * 2 + fh + 1]
                            if l < 3:
                                ht = htpool.tile([P, P], BF16,
                                                 tag=f"hT{fh}_{nt}", name=f"hT{fh}_{nt}")
                                nc.scalar.activation(
                                    out=ht[:], in_=pslice,
                                    func=mybir.ActivationFunctionType.Identity,
                                    bias=bcol, scale=1.0)
                                hT[(fh, nt)] = ht
                            else:
                                acol = prelua[:, fh:fh + 1]
                                neg = opool.tile([P, P], F32, tag="neg", name="neg")
                                nc.vector.tensor_scalar(
                                    out=neg[:], in0=pslice,
                                    scalar1=bcol, scalar2=0.0,
                                    op0=mybir.AluOpType.add,
                                    op1=mybir.AluOpType.min)
                                pos = opool.tile([P, P], F32, tag="pos", name="pos")
                                nc.vector.tensor_scalar(
                                    out=pos[:], in0=pslice,
                                    scalar1=bcol, scalar2=0.0,
                                    op0=mybir.AluOpType.add,
                                    op1=mybir.AluOpType.max)
                                nc.vector.tensor_scalar(
                                    out=neg[:], in0=neg[:],
                                    scalar1=acol, scalar2=None,
                                    op0=mybir.AluOpType.mult)
                                osb = opool.tile([P, P], F32, tag="osb", name="osb")
                                nc.vector.tensor_tensor(
                                    out=osb[:], in0=pos[:], in1=neg[:],
                                    op=mybir.AluOpType.add)
                                base = (fh * cfg.nb + nt) * P
                                nc.sync.dma_start(
                                    out=out_d[base:base + P, :], in_=osb[:])

            # ---- layer 1 (split halves so h0 gathers start mid-dense)
            a_hA = dram.tile([cfg.half, HID], BF16, tag="ahA", name="ahA")
            a_hB = dram.tile([cfg.npad - cfg.half, HID], BF16, tag="ahB", name="ahB")
            dense_full_l1(a_hA, a_hB)
            if debug_dense:
                rows = cfg.fh * cfg.nb * P
                nc.gpsimd.dma_start(out=out_d[:, :],
                                    in_=a_hA[0:rows, 0:P])
                nc.compile()
                return nc
            aggregate(0, (a_hA[:, :], a_hB[:, :]))
            # ---- layers 2..4
            for l in range(1, n_layers):
                a_shard = dram.tile([cfg.shard, HID], BF16, tag="ashard", name="ashard")
                dense_shard(l, a_shard)
                a_full = dram.tile([cfg.npad, HID], BF16, tag="afull", name="afull")
                nc.gpsimd.collective_compute(
                    "AllGather",
                    mybir.AluOpType.bypass,
                    ins=[a_shard[:].opt()],
                    outs=[a_full[:].opt()],
                    replica_groups=[list(range(cfg.ncores))],
                )
                aggregate(l, (a_full[0:cfg.half, :],
                              a_full[cfg.half:cfg.npad, :]))

            if n_layers < 4:
                # debug: dump hT tiles (post-bias h of layer n_layers) to out
                for nt in range(cfg.nb):
                    for fh in range(cfg.fh):
                        osb = opool.tile([P, P], F32, tag="osb", name="osbd")
                        nc.vector.tensor_copy(out=osb[:], in_=hT[(fh, nt)][:])
                        base = (fh * cfg.nb + nt) * P
                        nc.sync.dma_start(out=out_d[base:base + P, :], in_=osb[:])

    nc.compile()
    return nc


# ----------------------------------------------------------------- execution

def _make_in_maps(cfg, prep, x, Ws, bs, prelu_a):
    xts = _pack_xts(cfg, np.asarray(x, np.float32), prep["gslot"])
    wcat = _pack_wcat(cfg, Ws)
    biasp = _pack_bias(cfg, bs)
    prelup = np.zeros((P, 2), np.float32)
    prelup[:, 0] = prelu_a[:P]
    prelup[:, 1] = prelu_a[P:]
    iota = _iota_np()
    maps = []
    for c in range(cfg.ncores):
        maps.append({
            "gidx": prep["gidx"][c],
            "dstc": prep["dstc"][c],
            "normc": prep["normc"][c],
            "iota": iota,
            "wcat": wcat,
            "bias": biasp,
            "prelua": prelup,
            "xts": xts,
        })
    return maps


def _assemble_out(cfg, results, gslot):
    """results: list of per-core {'out': [fh*nb*128, 128]} -> y [n_nodes, hid]."""
    cores = []
    for c in range(cfg.ncores):
        o = results[c]["out"].reshape(cfg.fh, cfg.nb, P, P)
        # o[fh, nt, p, cpos] = h[f = fh*128+p, local slot = nt*128+cpos]
        oT = o.transpose(0, 2, 1, 3).reshape(cfg.hid, cfg.shard)
        cores.append(oT)
    yperm = np.concatenate(cores, axis=1).T        # [npad, hid]
    return np.ascontiguousarray(yperm[gslot]).astype(np.float32)


def run(cfg, x, edge_index, edge_weight, W1, b1, W2, b2, W3, b3, W4, b4,
        prelu_a, return_nc=False):
    prep = _preprocess(cfg, edge_index, edge_weight)
    nc = _build(cfg, prep["T"])
    in_maps = _make_in_maps(cfg, prep, x,
                            [np.asarray(W1, np.float32), np.asarray(W2, np.float32),
                             np.asarray(W3, np.float32), np.asarray(W4, np.float32)],
                            [np.asarray(b1, np.float32), np.asarray(b2, np.float32),
                             np.asarray(b3, np.float32), np.asarray(b4, np.float32)],
                            np.asarray(prelu_a, np.float32))
    res = run_bass_kernel_spmd(nc, in_maps, core_ids=list(range(cfg.ncores)))
    y = _assemble_out(cfg, res.results, prep["gslot"])
    if return_nc:
        return y, nc, in_maps
    return y


def kernel(x, edge_index, edge_weight, W1, b1, W2, b2, W3, b3, W4, b4, prelu_a):
    return run(CFG, x, edge_index, edge_weight,
               W1, b1, W2, b2, W3, b3, W4, b4, prelu_a)



# revision 10
# speedup vs baseline: 1.6207x; 1.6207x over previous
"""v2: 4-layer GCN encoder on 8 Trainium2 NeuronCores.

Key changes over v1:
  - Variable tiles per (block, src-half) bucket (max over cores, rank-matched
    block relabeling) -> ~12% fewer gather descriptors.
  - Self-loops removed from the edge stream; handled by an eye*dis diagonal
    matmul against the locally-kept dense output (asb tiles).
  - dis factorization: dense epilogue prescales rows by dis[src] (ACT scale),
    S values = ew * dis[dst] (host-precomputed) -> S build is 2 DVE ops per
    (group, half), no separate norm pass.
  - A/B region split: nodes laid out [all cores' A blocks | all cores' B
    blocks]; two smaller AllGathers per layer, emitted mid-aggregation of the
    previous layer so they overlap with gather/scatter work.
  - Layer-4 epilogue fused into one Prelu activation per (group, feat-half).
  - Deeper SWDGE descriptor ring (32KB scratch) so desc-gen overlaps drains.
  - PSUM: one full bank per (group, feat-half), double-buffered dense psum.
"""

import numpy as np
import ml_dtypes

import concourse.bacc as bacc
import concourse.mybir as mybir
import concourse.tile as tile
from concourse.bass_utils import run_bass_kernel_spmd

P = 128
BF16 = mybir.dt.bfloat16
F32 = mybir.dt.float32
I16 = mybir.dt.int16

NC = 8
NB = 49
NBA = 24
NBB = 25
G = 4
NGRP = (NB + G - 1) // G          # 13 groups (last has 1 block)
HALF_A = NC * NBA * P             # 24576
HALF_B = NC * NBB * P             # 25600
NPAD = HALF_A + HALF_B            # 50176
N_NODES = 50000
N_EDGES = 800000
IN_CH = 512
HID = 256
FC_IN = IN_CH // P                # 4
FH = HID // P                     # 2
SLAB = 4
NSLAB = NPAD // (SLAB * P)        # 98
CT = 8                            # max tiles per gather call (1024 idx cap)


def group_blocks():
    return [(g, list(range(g * G, min((g + 1) * G, NB)))) for g in range(NGRP)]


# ----------------------------------------------------------------- host prep

def _preprocess(edge_index, edge_weight):
    src0 = np.asarray(edge_index[0], dtype=np.int64)
    dst0 = np.asarray(edge_index[1], dtype=np.int64)
    ew0 = np.asarray(edge_weight, dtype=np.float32)
    N = N_NODES

    deg = np.bincount(dst0, weights=ew0.astype(np.float64), minlength=N)
    deg = (deg + 1.0).astype(np.float32)
    dis = (1.0 / np.sqrt(deg)).astype(np.float32)

    indeg = np.bincount(dst0, minlength=N)
    order = np.argsort(-indeg, kind="stable")
    nbins = NC * NB
    rank = np.empty(N, dtype=np.int64)
    rank[order] = np.arange(N)
    bin_of = rank % nbins
    core_of = bin_of % NC
    slot_of = bin_of // NC
    lane_of = rank // nbins
    assert lane_of.max() < P

    ecore = core_of[dst0]
    eslot = slot_of[dst0]
    cnt_cs = np.zeros((NC, NB), dtype=np.int64)
    np.add.at(cnt_cs, (ecore, eslot), 1)

    slot2blk = np.zeros((NC, NB), dtype=np.int64)
    for c in range(NC):
        ranked = np.argsort(-cnt_cs[c], kind="stable")
        a_slots = ranked[0::2][:NBA]
        a_set = set(a_slots.tolist())
        b_slots = [s for s in ranked if s not in a_set]
        for j, s in enumerate(a_slots):
            slot2blk[c, s] = j
        for j, s in enumerate(b_slots):
            slot2blk[c, s] = NBA + j
    blk_of = slot2blk[core_of, slot_of]

    base = np.where(
        blk_of < NBA,
        (core_of * NBA + blk_of) * P,
        HALF_A + (core_of * NBB + (blk_of - NBA)) * P,
    )
    gpos = base + lane_of

    gposL1 = np.empty((NC, N), dtype=np.int64)
    for c in range(NC):
        rot = (core_of - c) % NC
        baseL1 = np.where(
            blk_of < NBA,
            (rot * NBA + blk_of) * P,
            HALF_A + (rot * NBB + (blk_of - NBA)) * P,
        )
        gposL1[c] = baseL1 + lane_of

    h_of = (gpos[src0] >= HALF_A).astype(np.int64)
    ej = blk_of[dst0]
    ec = core_of[dst0]
    cnt = np.zeros((NC, NB, 2), dtype=np.int64)
    np.add.at(cnt, (ec, ej, h_of), 1)
    tiles = np.ceil(cnt.max(axis=0) / P).astype(np.int64)
    tiles = np.maximum(tiles, 1)

    tile_base = np.zeros((NB, 2), dtype=np.int64)
    call_ranges = []                                # (g, h, t0, t1)
    t = 0
    for g, blks in group_blocks():
        for h in (0, 1):
            t0 = t
            for j in blks:
                tile_base[j, h] = t
                t += tiles[j, h]
            call_ranges.append((g, h, t0, t))
    ntiles = int(t)
    nslots = ntiles * P

    ekey = (ec * NB + ej) * 2 + h_of
    esort = np.lexsort((gpos[src0], ekey))
    ks = ekey[esort]
    uniq, inv, counts = np.unique(ks, return_inverse=True, return_counts=True)
    starts = np.zeros_like(counts)
    starts[1:] = np.cumsum(counts)[:-1]
    rib = np.arange(len(esort)) - starts[inv]

    e_c = ec[esort]
    e_j = ej[esort]
    e_h = h_of[esort]
    e_src = src0[esort]
    e_dst = dst0[esort]
    e_w = ew0[esort]
    q = (tile_base[e_j, e_h] + rib // P) * P + rib % P

    gidx234 = np.zeros((NC, nslots), dtype=np.int16)
    gidxL1 = np.zeros((NC, nslots), dtype=np.int16)
    scmp = np.zeros((NC, P, ntiles), dtype=np.float32)
    sew = np.zeros((NC, P, ntiles), dtype=np.float32)

    idx234 = np.where(e_h == 1, gpos[e_src] - HALF_A, gpos[e_src])
    for c in range(NC):
        m = e_c == c
        qc = q[m]
        gidx234[c, qc] = idx234[m].astype(np.int16)
        iL1 = gposL1[c, e_src[m]]
        iL1 = np.where(e_h[m] == 1, iL1 - HALF_A, iL1)
        gidxL1[c, qc] = iL1.astype(np.int16)
        scmp[c, qc % P, qc // P] = lane_of[e_dst[m]].astype(np.float32)
        sew[c, qc % P, qc // P] = e_w[m] * dis[e_dst[m]]

    def wrap(a):
        w = a.reshape(NC, ntiles * 8, 16).transpose(0, 2, 1)
        return np.ascontiguousarray(np.tile(w, (1, 8, 1)))

    # discL1 [NC, P, 392]: dis per (lane, L1-dense block index nt)
    discL1 = np.zeros((NC, P, NC * NB), dtype=np.float32)
    # nt for node v on core c: A: rot*24+j (blocks 0..191), B: 192 + rot*25+(j-24)
    for c in range(NC):
        rot = (core_of - c) % NC
        nt = np.where(
            blk_of < NBA, rot * NBA + blk_of,
            NC * NBA + rot * NBB + (blk_of - NBA),
        )
        discL1[c, lane_of, nt] = dis

    # eyedis [NC, P, NB*P]: (p==d) * dis[node(c, j, d)]
    eyedis = np.zeros((NC, P, NB * P), dtype=np.float32)
    for c in range(NC):
        m = core_of == c
        eyedis[c, lane_of[m], blk_of[m] * P + lane_of[m]] = dis[m]

    nid = np.full((NC, NB, P), -1, dtype=np.int64)
    nid[core_of, blk_of, lane_of] = np.arange(N)

    return dict(
        tiles=tiles, tile_base=tile_base, ntiles=ntiles, nslots=nslots,
        call_ranges=call_ranges,
        gidx234=wrap(gidx234), gidxL1=wrap(gidxL1),
        scmp=scmp.astype(ml_dtypes.bfloat16), sew=sew.astype(ml_dtypes.bfloat16),
        discL1=discL1, eyedis=eyedis.astype(ml_dtypes.bfloat16),
        dis=dis, gpos=gpos, gposL1=gposL1, nid=nid,
    )


def _pack_xts(x, gposL1_c):
    """Per-core permuted/transposed/slab-tiled x for L1 lhsT streaming."""
    xpad = np.zeros((NPAD, IN_CH), dtype=np.float32)
    xpad[gposL1_c] = x
    a = xpad.T.reshape(FC_IN, P, NSLAB, SLAB, P)
    a = a.transpose(0, 2, 1, 3, 4).reshape(FC_IN, NSLAB, P, SLAB * P)
    return np.ascontiguousarray(a.astype(ml_dtypes.bfloat16)).reshape(
        FC_IN * NSLAB * P, SLAB * P)


def _pack_wcat(Ws):
    cols = []
    for Wl in Ws:
        k = Wl.shape[0]
        for fc in range(k // P):
            cols.append(Wl[fc * P:(fc + 1) * P, :])
    return np.concatenate(cols, axis=1).astype(ml_dtypes.bfloat16)


def _pack_bias(bs):
    out = np.zeros((P, 8), dtype=np.float32)
    for l, b in enumerate(bs):
        for fh in range(FH):
            out[:, l * 2 + fh] = b[fh * P:(fh + 1) * P]
    return out


# ----------------------------------------------------------------- builder

def _build(prep):
    tiles = prep["tiles"]
    tile_base = prep["tile_base"]
    ntiles = prep["ntiles"]
    call_ranges = prep["call_ranges"]
    maxgh = max(t1 - t0 for (_, _, t0, t1) in call_ranges)

    nc = bacc.Bacc("TRN2", target_bir_lowering=False, debug=False,
                   num_devices=NC, num_swdge_queues=4,
                   dynamic_dma_scratch_size=2 * 16384)
    qctr = [0]

    gidxL1_d = nc.dram_tensor("gidxL1", [P, ntiles * 8], I16, kind="ExternalInput")
    gidx234_d = nc.dram_tensor("gidx234", [P, ntiles * 8], I16, kind="ExternalInput")
    scmp_d = nc.dram_tensor("scmp", [P, ntiles], BF16, kind="ExternalInput")
    sew_d = nc.dram_tensor("sew", [P, ntiles], BF16, kind="ExternalInput")
    iota_d = nc.dram_tensor("iota", [P, P], BF16, kind="ExternalInput")
    eyedis_d = nc.dram_tensor("eyedis", [P, NB * P], BF16, kind="ExternalInput")
    wcat_cols = (FC_IN + 3 * FH) * HID
    wcat_d = nc.dram_tensor("wcat", [P, wcat_cols], BF16, kind="ExternalInput")
    bias_d = nc.dram_tensor("bias", [P, 8], F32, kind="ExternalInput")
    prelu_d = nc.dram_tensor("prelua", [P, 2], F32, kind="ExternalInput")
    discL1_d = nc.dram_tensor("discL1", [P, NC * NB], F32, kind="ExternalInput")
    xts_d = nc.dram_tensor("xts", [FC_IN * NSLAB * P, SLAB * P], BF16,
                           kind="ExternalInput")
    out_d = nc.dram_tensor("out", [FH * NB * P, P], F32, kind="ExternalOutput")

    w_off = {}
    off = 0
    for l in range(4):
        k = FC_IN if l == 0 else FH
        for fc in range(k):
            w_off[(l, fc)] = off
            off += HID

    # nt -> local block j for L1 dense (core's own blocks come first per region)
    def local_j(nt):
        if nt < NBA:
            return nt
        if NC * NBA <= nt < NC * NBA + NBB:
            return NBA + (nt - NC * NBA)
        return None

    with tile.TileContext(nc) as tc:
        with (
            tc.tile_pool(name="res", bufs=1) as res,
            tc.tile_pool(name="xpool", bufs=2) as xpool,
            tc.tile_pool(name="apool", bufs=3) as apool,
            tc.tile_pool(name="akeep", bufs=1) as akeep,
            tc.tile_pool(name="mpool", bufs=2) as mpool,
            tc.tile_pool(name="spool", bufs=2) as spool,
            tc.tile_pool(name="htpool", bufs=1) as htpool,
            tc.tile_pool(name="opool", bufs=4) as opool,
            tc.tile_pool(name="ppool", bufs=6, space="PSUM") as ppool,
            tc.tile_pool(name="dpsum", bufs=2, space="PSUM") as dpsum,
            tc.tile_pool(name="dram", bufs=2, space="DRAM") as dram,
            tc.tile_pool(name="dram1", bufs=1, space="DRAM") as dram1,
        ):
            # resident loads
            gidxL1 = res.tile([P, ntiles * 8], I16)
            nc.sync.dma_start(out=gidxL1[:], in_=gidxL1_d[:])
            gidx234 = res.tile([P, ntiles * 8], I16)
            nc.sync.dma_start(out=gidx234[:], in_=gidx234_d[:])
            scmp = res.tile([P, ntiles], BF16)
            nc.sync.dma_start(out=scmp[:], in_=scmp_d[:])
            sew = res.tile([P, ntiles], BF16)
            nc.sync.dma_start(out=sew[:], in_=sew_d[:])
            iota = res.tile([P, P], BF16)
            nc.sync.dma_start(out=iota[:], in_=iota_d[:])
            eyedis = res.tile([P, NB * P], BF16)
            nc.sync.dma_start(out=eyedis[:], in_=eyedis_d[:])
            wcat = res.tile([P, wcat_cols], BF16)
            nc.sync.dma_start(out=wcat[:], in_=wcat_d[:])
            bias = res.tile([P, 8], F32)
            nc.sync.dma_start(out=bias[:], in_=bias_d[:])
            prelua = res.tile([P, 2], F32)
            nc.sync.dma_start(out=prelua[:], in_=prelu_d[:])
            discL1 = res.tile([P, NC * NB], F32)
            nc.sync.dma_start(out=discL1[:], in_=discL1_d[:])

            a_hA = dram1.tile([HALF_A, HID], BF16, tag="ahA", name="ahA")
            a_hB = dram1.tile([HALF_B, HID], BF16, tag="ahB", name="ahB")

            asb_local = {}

            # ---------------- L1 dense (full, core-private order)
            for s in range(NSLAB):
                xsl = [xpool.tile([P, SLAB * P], BF16, tag=f"x{fc}",
                                  name=f"xsl{fc}") for fc in range(FC_IN)]
                for fc in range(FC_IN):
                    b0 = (fc * NSLAB + s) * P
                    nc.sync.dma_start(out=xsl[fc][:], in_=xts_d[b0:b0 + P, :])
                for t in range(SLAB):
                    nt = s * SLAB + t
                    pd_ = dpsum.tile([P, HID], F32, tag="dps", name="pd1")
                    for fc in range(FC_IN):
                        nc.tensor.matmul(
                            out=pd_[:],
                            lhsT=xsl[fc][:, t * P:(t + 1) * P],
                            rhs=wcat[:, w_off[(0, fc)]:w_off[(0, fc)] + HID],
                            start=(fc == 0), stop=(fc == FC_IN - 1))
                    lj = local_j(nt)
                    if lj is not None:
                        asb = akeep.tile([P, HID], BF16, tag=f"asb{lj}",
                                         name=f"asbk{lj}")
                        asb_local[lj] = asb
                    else:
                        asb = apool.tile([P, HID], BF16, tag="asb", name="asb1")
                    nc.scalar.activation(
                        out=asb[:], in_=pd_[:],
                        func=mybir.ActivationFunctionType.Identity,
                        bias=0.0, scale=discL1[:, nt:nt + 1])
                    if nt < NC * NBA:
                        nc.sync.dma_start(
                            out=a_hA[nt * P:(nt + 1) * P, :], in_=asb[:])
                    else:
                        b0 = (nt - NC * NBA) * P
                        nc.sync.dma_start(out=a_hB[b0:b0 + P, :], in_=asb[:])

            def gathers(gi, h, gidx, srcA, srcB):
                """Gather all tiles of (group gi, half h) into an M tile."""
                (g, hh, t0, t1) = call_ranges[gi * 2 + h]
                assert g == gi and hh == h
                n = t1 - t0
                M = mpool.tile([P, maxgh * HID], BF16, tag="M", name="M")
                ncalls = (n + CT - 1) // CT
                chunk = (n + ncalls - 1) // ncalls
                src = srcA if h == 0 else srcB
                for k0 in range(0, n, chunk):
                    k1 = min(k0 + chunk, n)
                    nt_ = k1 - k0
                    nc.gpsimd.dma_gather(
                        out_ap=M[:, k0 * HID:k1 * HID].rearrange(
                            "p (t e) -> p t e", e=HID),
                        in_ap=src,
                        idxs_ap=gidx[:, (t0 + k0) * 8:(t0 + k1) * 8],
                        num_idxs=nt_ * P,
                        num_idxs_reg=nt_ * P,
                        elem_size=HID,
                        queue_num=qctr[0] % 4,
                    )
                    qctr[0] += 1
                S = spool.tile([P, maxgh * P], BF16, tag="S", name="S")
                s3 = S[:, :n * P].rearrange("p (t e) -> p t e", e=P)
                iob = iota[:].rearrange("p (o e) -> p o e", o=1).broadcast_to(
                    [P, n, P])
                nc.vector.tensor_tensor(
                    out=s3, in0=iob, in1=scmp[:, t0:t1].to_broadcast([P, n, P]),
                    op=mybir.AluOpType.is_equal)
                nc.vector.tensor_tensor(
                    out=s3, in0=s3, in1=sew[:, t0:t1].to_broadcast([P, n, P]),
                    op=mybir.AluOpType.mult)
                return M, S, t0

            def aggregate(layer, gidx, srcA, srcB, post_group=None):
                for gi, blks in group_blocks():
                    pbf = [ppool.tile([P, G * P], F32, tag="pbf", name="pbf")
                           for _ in range(FH)]
                    # One PSUM accumulation group per pbf[fh] bank: start on
                    # the first matmul (start zeroes the whole 2KB zero
                    # region), stop on the last emitted matmul for the bank.
                    for bj, j in enumerate(blks):
                        for fh in range(FH):
                            nc.tensor.matmul(
                                out=pbf[fh][:, bj * P:(bj + 1) * P],
                                lhsT=asb_local[j][:, fh * P:(fh + 1) * P],
                                rhs=eyedis[:, j * P:(j + 1) * P],
                                start=(bj == 0), stop=False)
                    for h in (0, 1):
                        M, S, t0 = gathers(gi, h, gidx, srcA, srcB)
                        for bj, j in enumerate(blks):
                            for t in range(tiles[j, h]):
                                tl = tile_base[j, h] - t0 + t
                                last = (h == 1 and bj == len(blks) - 1
                                        and t == tiles[j, 1] - 1)
                                for fh in range(FH):
                                    nc.tensor.matmul(
                                        out=pbf[fh][:, bj * P:(bj + 1) * P],
                                        lhsT=M[:, tl * HID + fh * P:
                                               tl * HID + (fh + 1) * P],
                                        rhs=S[:, tl * P:(tl + 1) * P],
                                        start=False,
                                        stop=last)
                    gw = len(blks)
                    if layer < 3:
                        for fh in range(FH):
                            ht = htpool.tile([P, G * P], BF16,
                                             tag=f"ht{gi}_{fh}",
                                             name=f"ht{gi}_{fh}")
                            nc.scalar.activation(
                                out=ht[:, :gw * P], in_=pbf[fh][:, :gw * P],
                                func=mybir.ActivationFunctionType.Identity,
                                bias=bias[:, layer * 2 + fh:layer * 2 + fh + 1],
                                scale=1.0)
                            hT[(gi, fh)] = ht
                    else:
                        for fh in range(FH):
                            osb = opool.tile([P, G * P], F32, tag="osb",
                                             name="osb")
                            nc.scalar.activation(
                                out=osb[:, :gw * P], in_=pbf[fh][:, :gw * P],
                                func=mybir.ActivationFunctionType.Prelu,
                                bias=bias[:, 6 + fh:7 + fh],
                                scale=1.0,
                                alpha=prelua[:, fh:fh + 1])
                            for bj, j in enumerate(blks):
                                b0 = (fh * NB + j) * P
                                nc.sync.dma_start(
                                    out=out_d[b0:b0 + P, :],
                                    in_=osb[:, bj * P:(bj + 1) * P])
                    if post_group is not None:
                        post_group(gi)

            def dense_region(layer, blks, a_sh, row0):
                """Dense projection for blocks blks of layer `layer` (1-based
                weight index), writing bf16*dis rows into a_sh."""
                for j in blks:
                    gi, bj = j // G, j % G
                    pd_ = dpsum.tile([P, HID], F32, tag="dps", name="pd2")
                    for fc in range(FH):
                        nc.tensor.matmul(
                            out=pd_[:],
                            lhsT=hT[(gi, fc)][:, bj * P:(bj + 1) * P],
                            rhs=wcat[:, w_off[(layer, fc)]:
                                     w_off[(layer, fc)] + HID],
                            start=(fc == 0), stop=(fc == FH - 1))
                    asb = akeep.tile([P, HID], BF16, tag=f"asb{j}",
                                     name=f"asbk{j}")
                    asb_local[j] = asb
                    ntl = j if j < NBA else NC * NBA + (j - NBA)
                    nc.scalar.activation(
                        out=asb[:], in_=pd_[:],
                        func=mybir.ActivationFunctionType.Identity,
                        bias=0.0, scale=discL1[:, ntl:ntl + 1])
                    b0 = (j - row0) * P
                    nc.sync.dma_start(out=a_sh[b0:b0 + P, :], in_=asb[:])

            hT = {}
            srcA, srcB = a_hA[:, :], a_hB[:, :]
            gidx_cur = gidxL1
            for layer in range(4):
                if layer < 3:
                    nextA = dram.tile([NBA * P, HID], BF16, tag="shA", name="shA")
                    nextB = dram.tile([NBB * P, HID], BF16, tag="shB", name="shB")
                    fullA = dram.tile([HALF_A, HID], BF16, tag="fA", name="fA")
                    fullB = dram.tile([HALF_B, HID], BF16, tag="fB", name="fB")

                    def post(gi, nextA=nextA, nextB=nextB, fullA=fullA,
                             fullB=fullB, layer=layer):
                        if gi == NBA // G - 1:          # blocks 0..23 done
                            dense_region(layer + 1, range(NBA), nextA, 0)
                            nc.gpsimd.collective_compute(
                                "AllGather", mybir.AluOpType.bypass,
                                ins=[nextA[:].opt()], outs=[fullA[:].opt()],
                                replica_groups=[list(range(NC))])
                        if gi == NGRP - 1:              # blocks 24..48 done
                            dense_region(layer + 1, range(NBA, NB), nextB, NBA)
                            nc.gpsimd.collective_compute(
                                "AllGather", mybir.AluOpType.bypass,
                                ins=[nextB[:].opt()], outs=[fullB[:].opt()],
                                replica_groups=[list(range(NC))])
                    aggregate(layer, gidx_cur, srcA, srcB, post_group=post)
                    srcA, srcB = fullA[:, :], fullB[:, :]
                    gidx_cur = gidx234
                else:
                    aggregate(layer, gidx_cur, srcA, srcB)

    nc.compile()
    return nc


# ----------------------------------------------------------------- execution

def _iota_np():
    return np.tile(np.arange(P, dtype=np.float32)[None, :], (P, 1)).astype(
        ml_dtypes.bfloat16)


def _make_in_maps(prep, x, Ws, bs, prelu_a):
    wcat = _pack_wcat(Ws)
    biasp = _pack_bias(bs)
    prelup = np.zeros((P, 2), np.float32)
    prelup[:, 0] = prelu_a[:P]
    prelup[:, 1] = prelu_a[P:]
    iota = _iota_np()
    maps = []
    xf = np.asarray(x, np.float32)
    for c in range(NC):
        maps.append({
            "gidxL1": prep["gidxL1"][c],
            "gidx234": prep["gidx234"][c],
            "scmp": prep["scmp"][c],
            "sew": prep["sew"][c],
            "iota": iota,
            "eyedis": prep["eyedis"][c],
            "wcat": wcat,
            "bias": biasp,
            "prelua": prelup,
            "discL1": prep["discL1"][c],
            "xts": _pack_xts(xf, prep["gposL1"][c]),
        })
    return maps


def _assemble_out(prep, results):
    y = np.zeros((N_NODES, HID), dtype=np.float32)
    nid = prep["nid"]
    for c in range(NC):
        o = results[c]["out"].reshape(FH, NB, P, P)
        # o[fh, j, p, lane] = h[feat = fh*128+p, node (c, j, lane)]
        for fh in range(FH):
            for j in range(NB):
                nids = nid[c, j]
                ok = nids >= 0
                y[nids[ok], fh * P:(fh + 1) * P] = o[fh, j, :, ok]
    return y


def run(x, edge_index, edge_weight, W1, b1, W2, b2, W3, b3, W4, b4, prelu_a):
    prep = _preprocess(edge_index, edge_weight)
    nc = _build(prep)
    in_maps = _make_in_maps(
        prep, x,
        [np.asarray(W1, np.float32), np.asarray(W2, np.float32),
         np.asarray(W3, np.float32), np.asarray(W4, np.float32)],
        [np.asarray(b1, np.float32), np.asarray(b2, np.float32),
         np.asarray(b3, np.float32), np.asarray(b4, np.float32)],
        np.asarray(prelu_a, np.float32))
    res = run_bass_kernel_spmd(nc, in_maps, core_ids=list(range(NC)))
    return _assemble_out(prep, res.results).astype(np.float32)


def kernel(x, edge_index, edge_weight, W1, b1, W2, b2, W3, b3, W4, b4, prelu_a):
    return run(x, edge_index, edge_weight,
               W1, b1, W2, b2, W3, b3, W4, b4, prelu_a)


# revision 11
# speedup vs baseline: 1.8518x; 1.1425x over previous
"""v3: 4-layer GCN encoder on 8 Trainium2 NeuronCores.

Changes over v2:
  - Layer-1 dense is sharded like layers 2-4 (own 49 blocks only) and
    AllGathered; removes the replicated 50k x 512 projection (which measured
    ~600us serial on PE) and the per-core permuted x layout / second index
    table.
  - Software-pipelined aggregation: per stage s, gather h0(s) is emitted
    before mm-h0(s-1), gather h1(s-1), mm-h1(s-2), epilogue(s-2). The h1
    gather of a group trails its h0 gather by two phases so the Pool engine
    keeps working while AG-B of the current layer is still landing.
  - Single gather index table for all layers (global A|B node layout).
"""

import numpy as np
import ml_dtypes

import concourse.bacc as bacc
import concourse.mybir as mybir
import concourse.tile as tile
from concourse.bass_utils import run_bass_kernel_spmd

P = 128
BF16 = mybir.dt.bfloat16
F32 = mybir.dt.float32
I16 = mybir.dt.int16

NC = 8
NB = 49
NBA = 24
NBB = 25
G = 4
NGRP = (NB + G - 1) // G          # 13 groups (last has 1 block)
NGA = NBA // G                    # 6 groups cover the A blocks exactly
HALF_A = NC * NBA * P             # 24576
HALF_B = NC * NBB * P             # 25600
NPAD = HALF_A + HALF_B            # 50176
N_NODES = 50000
N_EDGES = 800000
IN_CH = 512
HID = 256
FC_IN = IN_CH // P                # 4
FH = HID // P                     # 2
CT = 8                            # max tiles per gather call (1024 idx cap)


def group_blocks():
    return [(g, list(range(g * G, min((g + 1) * G, NB)))) for g in range(NGRP)]


# ----------------------------------------------------------------- host prep

def _preprocess(edge_index, edge_weight):
    src0 = np.asarray(edge_index[0], dtype=np.int64)
    dst0 = np.asarray(edge_index[1], dtype=np.int64)
    ew0 = np.asarray(edge_weight, dtype=np.float32)
    N = N_NODES

    deg = np.bincount(dst0, weights=ew0.astype(np.float64), minlength=N)
    deg = (deg + 1.0).astype(np.float32)
    dis = (1.0 / np.sqrt(deg)).astype(np.float32)

    indeg = np.bincount(dst0, minlength=N)
    order = np.argsort(-indeg, kind="stable")
    nbins = NC * NB
    rank = np.empty(N, dtype=np.int64)
    rank[order] = np.arange(N)
    bin_of = rank % nbins
    core_of = bin_of % NC
    slot_of = bin_of // NC
    lane_of = rank // nbins
    assert lane_of.max() < P

    ecore = core_of[dst0]
    eslot = slot_of[dst0]
    cnt_cs = np.zeros((NC, NB), dtype=np.int64)
    np.add.at(cnt_cs, (ecore, eslot), 1)

    slot2blk = np.zeros((NC, NB), dtype=np.int64)
    for c in range(NC):
        ranked = np.argsort(-cnt_cs[c], kind="stable")
        a_slots = ranked[0::2][:NBA]
        a_set = set(a_slots.tolist())
        b_slots = [s for s in ranked if s not in a_set]
        for j, s in enumerate(a_slots):
            slot2blk[c, s] = j
        for j, s in enumerate(b_slots):
            slot2blk[c, s] = NBA + j
    blk_of = slot2blk[core_of, slot_of]

    base = np.where(
        blk_of < NBA,
        (core_of * NBA + blk_of) * P,
        HALF_A + (core_of * NBB + (blk_of - NBA)) * P,
    )
    gpos = base + lane_of

    h_of = (gpos[src0] >= HALF_A).astype(np.int64)
    ej = blk_of[dst0]
    ec = core_of[dst0]
    cnt = np.zeros((NC, NB, 2), dtype=np.int64)
    np.add.at(cnt, (ec, ej, h_of), 1)
    tiles = np.ceil(cnt.max(axis=0) / P).astype(np.int64)
    tiles = np.maximum(tiles, 1)

    tile_base = np.zeros((NB, 2), dtype=np.int64)
    call_ranges = []                                # (g, h, t0, t1)
    t = 0
    for g, blks in group_blocks():
        for h in (0, 1):
            t0 = t
            for j in blks:
                tile_base[j, h] = t
                t += tiles[j, h]
            call_ranges.append((g, h, t0, t))
    ntiles = int(t)
    nslots = ntiles * P

    ekey = (ec * NB + ej) * 2 + h_of
    esort = np.lexsort((gpos[src0], ekey))
    ks = ekey[esort]
    uniq, inv, counts = np.unique(ks, return_inverse=True, return_counts=True)
    starts = np.zeros_like(counts)
    starts[1:] = np.cumsum(counts)[:-1]
    rib = np.arange(len(esort)) - starts[inv]

    e_c = ec[esort]
    e_j = ej[esort]
    e_h = h_of[esort]
    e_src = src0[esort]
    e_dst = dst0[esort]
    e_w = ew0[esort]
    q = (tile_base[e_j, e_h] + rib // P) * P + rib % P

    gidx = np.zeros((NC, nslots), dtype=np.int16)
    scmp = np.zeros((NC, P, ntiles), dtype=np.float32)
    sew = np.zeros((NC, P, ntiles), dtype=np.float32)

    idxv = np.where(e_h == 1, gpos[e_src] - HALF_A, gpos[e_src])
    for c in range(NC):
        m = e_c == c
        qc = q[m]
        gidx[c, qc] = idxv[m].astype(np.int16)
        scmp[c, qc % P, qc // P] = lane_of[e_dst[m]].astype(np.float32)
        sew[c, qc % P, qc // P] = e_w[m] * dis[e_dst[m]]

    def wrap(a):
        w = a.reshape(NC, ntiles * 8, 16).transpose(0, 2, 1)
        return np.ascontiguousarray(np.tile(w, (1, 8, 1)))

    # disc [NC, P, NB]: dis per (lane, own block)
    disc = np.zeros((NC, P, NB), dtype=np.float32)
    disc[core_of, lane_of, blk_of] = dis

    # eyedis [NC, P, NB*P]: (p==d) * dis[node(c, j, d)]
    eyedis = np.zeros((NC, P, NB * P), dtype=np.float32)
    for c in range(NC):
        m = core_of == c
        eyedis[c, lane_of[m], blk_of[m] * P + lane_of[m]] = dis[m]

    nid = np.full((NC, NB, P), -1, dtype=np.int64)
    nid[core_of, blk_of, lane_of] = np.arange(N)

    return dict(
        tiles=tiles, tile_base=tile_base, ntiles=ntiles, nslots=nslots,
        call_ranges=call_ranges,
        gidx=wrap(gidx),
        scmp=scmp.astype(ml_dtypes.bfloat16), sew=sew.astype(ml_dtypes.bfloat16),
        disc=disc, eyedis=eyedis.astype(ml_dtypes.bfloat16),
        dis=dis, gpos=gpos, nid=nid,
    )


def _pack_xown(x, nid_c):
    """x^T tiles for the core's own 49 blocks: rows (fc*NB+j)*P + p, col=lane.
    xo[(fc*NB+j)*P + p, lane] = x[node(c, j, lane), fc*P + p]."""
    out = np.zeros((FC_IN * NB * P, P), dtype=np.float32)
    for j in range(NB):
        nods = nid_c[j]
        ok = nods >= 0
        xv = x[nods[ok]]                        # [nok, IN_CH]
        for fc in range(FC_IN):
            out[(fc * NB + j) * P:(fc * NB + j) * P + P, ok] = \
                xv[:, fc * P:(fc + 1) * P].T
    return out.astype(ml_dtypes.bfloat16)


def _pack_wcat(Ws):
    cols = []
    for Wl in Ws:
        k = Wl.shape[0]
        for fc in range(k // P):
            cols.append(Wl[fc * P:(fc + 1) * P, :])
    return np.concatenate(cols, axis=1).astype(ml_dtypes.bfloat16)


def _pack_bias(bs):
    out = np.zeros((P, 8), dtype=np.float32)
    for l, b in enumerate(bs):
        for fh in range(FH):
            out[:, l * 2 + fh] = b[fh * P:(fh + 1) * P]
    return out


# ----------------------------------------------------------------- builder

def _build(prep):
    tiles = prep["tiles"]
    tile_base = prep["tile_base"]
    ntiles = prep["ntiles"]
    call_ranges = prep["call_ranges"]
    maxgh = max(t1 - t0 for (_, _, t0, t1) in call_ranges)

    nc = bacc.Bacc("TRN2", target_bir_lowering=False, debug=False,
                   num_devices=NC, num_swdge_queues=4,
                   dynamic_dma_scratch_size=2 * 16384)
    qctr = [0]

    gidx_d = nc.dram_tensor("gidx", [P, ntiles * 8], I16, kind="ExternalInput")
    scmp_d = nc.dram_tensor("scmp", [P, ntiles], BF16, kind="ExternalInput")
    sew_d = nc.dram_tensor("sew", [P, ntiles], BF16, kind="ExternalInput")
    iota_d = nc.dram_tensor("iota", [P, P], BF16, kind="ExternalInput")
    eyedis_d = nc.dram_tensor("eyedis", [P, NB * P], BF16, kind="ExternalInput")
    wcat_cols = (FC_IN + 3 * FH) * HID
    wcat_d = nc.dram_tensor("wcat", [P, wcat_cols], BF16, kind="ExternalInput")
    bias_d = nc.dram_tensor("bias", [P, 8], F32, kind="ExternalInput")
    prelu_d = nc.dram_tensor("prelua", [P, 2], F32, kind="ExternalInput")
    disc_d = nc.dram_tensor("disc", [P, NB], F32, kind="ExternalInput")
    xown_d = nc.dram_tensor("xown", [FC_IN * NB * P, P], BF16,
                            kind="ExternalInput")
    out_d = nc.dram_tensor("out", [FH * NB * P, P], F32, kind="ExternalOutput")

    w_off = {}
    off = 0
    for l in range(4):
        k = FC_IN if l == 0 else FH
        for fc in range(k):
            w_off[(l, fc)] = off
            off += HID

    with tile.TileContext(nc) as tc:
        with (
            tc.tile_pool(name="res", bufs=1) as res,
            tc.tile_pool(name="xpool", bufs=8) as xpool,
            tc.tile_pool(name="akeep", bufs=1) as akeep,
            tc.tile_pool(name="mpool", bufs=3) as mpool,
            tc.tile_pool(name="spool", bufs=3) as spool,
            tc.tile_pool(name="htpool", bufs=1) as htpool,
            tc.tile_pool(name="opool", bufs=4) as opool,
            tc.tile_pool(name="ppool", bufs=6, space="PSUM") as ppool,
            tc.tile_pool(name="dpsum", bufs=2, space="PSUM") as dpsum,
            tc.tile_pool(name="dram", bufs=2, space="DRAM") as dram,
        ):
            # resident loads
            gidx = res.tile([P, ntiles * 8], I16)
            nc.sync.dma_start(out=gidx[:], in_=gidx_d[:])
            scmp = res.tile([P, ntiles], BF16)
            nc.sync.dma_start(out=scmp[:], in_=scmp_d[:])
            sew = res.tile([P, ntiles], BF16)
            nc.sync.dma_start(out=sew[:], in_=sew_d[:])
            iota = res.tile([P, P], BF16)
            nc.sync.dma_start(out=iota[:], in_=iota_d[:])
            eyedis = res.tile([P, NB * P], BF16)
            nc.sync.dma_start(out=eyedis[:], in_=eyedis_d[:])
            wcat = res.tile([P, wcat_cols], BF16)
            nc.sync.dma_start(out=wcat[:], in_=wcat_d[:])
            bias = res.tile([P, 8], F32)
            nc.sync.dma_start(out=bias[:], in_=bias_d[:])
            prelua = res.tile([P, 2], F32)
            nc.sync.dma_start(out=prelua[:], in_=prelu_d[:])
            disc = res.tile([P, NB], F32)
            nc.sync.dma_start(out=disc[:], in_=disc_d[:])

            asb_local = {}
            hT = {}

            def dense_block(layer, j, a_shA, a_shB):
                """Dense projection for own block j of `layer` (weight index),
                dis-prescaled bf16 into akeep + the shard DRAM chunk."""
                pd_ = dpsum.tile([P, HID], F32, tag="dps", name="pd")
                if layer == 0:
                    xbl = [xpool.tile([P, P], BF16, tag=f"xo{fc}",
                                      name=f"xo{fc}") for fc in range(FC_IN)]
                    for fc in range(FC_IN):
                        b0 = (fc * NB + j) * P
                        nc.sync.dma_start(out=xbl[fc][:],
                                          in_=xown_d[b0:b0 + P, :])
                    for fc in range(FC_IN):
                        nc.tensor.matmul(
                            out=pd_[:], lhsT=xbl[fc][:],
                            rhs=wcat[:, w_off[(0, fc)]:w_off[(0, fc)] + HID],
                            start=(fc == 0), stop=(fc == FC_IN - 1))
                else:
                    gi, bj = j // G, j % G
                    for fc in range(FH):
                        nc.tensor.matmul(
                            out=pd_[:],
                            lhsT=hT[(gi, fc)][:, bj * P:(bj + 1) * P],
                            rhs=wcat[:, w_off[(layer, fc)]:
                                     w_off[(layer, fc)] + HID],
                            start=(fc == 0), stop=(fc == FH - 1))
                asb = akeep.tile([P, HID], BF16, tag=f"asb{j}", name=f"asb{j}")
                asb_local[j] = asb
                nc.scalar.activation(
                    out=asb[:], in_=pd_[:],
                    func=mybir.ActivationFunctionType.Identity,
                    bias=0.0, scale=disc[:, j:j + 1])
                if j < NBA:
                    nc.sync.dma_start(
                        out=a_shA[j * P:(j + 1) * P, :], in_=asb[:])
                else:
                    b0 = (j - NBA) * P
                    nc.sync.dma_start(out=a_shB[b0:b0 + P, :], in_=asb[:])

            def gathers(gi, h, srcA, srcB):
                (g, hh, t0, t1) = call_ranges[gi * 2 + h]
                n = t1 - t0
                M = mpool.tile([P, maxgh * HID], BF16, tag="M", name="M")
                ncalls = (n + CT - 1) // CT
                chunk = (n + ncalls - 1) // ncalls
                src = srcA if h == 0 else srcB
                for k0 in range(0, n, chunk):
                    k1 = min(k0 + chunk, n)
                    nt_ = k1 - k0
                    nc.gpsimd.dma_gather(
                        out_ap=M[:, k0 * HID:k1 * HID].rearrange(
                            "p (t e) -> p t e", e=HID),
                        in_ap=src,
                        idxs_ap=gidx[:, (t0 + k0) * 8:(t0 + k1) * 8],
                        num_idxs=nt_ * P,
                        num_idxs_reg=nt_ * P,
                        elem_size=HID,
                        queue_num=qctr[0] % 4,
                    )
                    qctr[0] += 1
                S = spool.tile([P, maxgh * P], BF16, tag="S", name="S")
                s3 = S[:, :n * P].rearrange("p (t e) -> p t e", e=P)
                iob = iota[:].rearrange("p (o e) -> p o e", o=1).broadcast_to(
                    [P, n, P])
                nc.vector.tensor_tensor(
                    out=s3, in0=iob, in1=scmp[:, t0:t1].to_broadcast([P, n, P]),
                    op=mybir.AluOpType.is_equal)
                nc.vector.tensor_tensor(
                    out=s3, in0=s3, in1=sew[:, t0:t1].to_broadcast([P, n, P]),
                    op=mybir.AluOpType.mult)
                return M, S, t0

            def mm_half(gi, h, pbf, M, S, t0, blks):
                for bj, j in enumerate(blks):
                    for t in range(tiles[j, h]):
                        tl = tile_base[j, h] - t0 + t
                        last = (h == 1 and bj == len(blks) - 1
                                and t == tiles[j, 1] - 1)
                        for fh in range(FH):
                            nc.tensor.matmul(
                                out=pbf[fh][:, bj * P:(bj + 1) * P],
                                lhsT=M[:, tl * HID + fh * P:
                                       tl * HID + (fh + 1) * P],
                                rhs=S[:, tl * P:(tl + 1) * P],
                                start=False, stop=last)

            def epilogue(layer, gi, pbf, blks):
                gw = len(blks)
                if layer < 3:
                    for fh in range(FH):
                        ht = htpool.tile([P, G * P], BF16,
                                         tag=f"ht{gi}_{fh}", name=f"ht{gi}_{fh}")
                        nc.scalar.activation(
                            out=ht[:, :gw * P], in_=pbf[fh][:, :gw * P],
                            func=mybir.ActivationFunctionType.Identity,
                            bias=bias[:, layer * 2 + fh:layer * 2 + fh + 1],
                            scale=1.0)
                        hT[(gi, fh)] = ht
                else:
                    for fh in range(FH):
                        osb = opool.tile([P, G * P], F32, tag="osb", name="osb")
                        nc.scalar.activation(
                            out=osb[:, :gw * P], in_=pbf[fh][:, :gw * P],
                            func=mybir.ActivationFunctionType.Prelu,
                            bias=bias[:, 6 + fh:7 + fh],
                            scale=1.0, alpha=prelua[:, fh:fh + 1])
                        for bj, j in enumerate(blks):
                            b0 = (fh * NB + j) * P
                            nc.sync.dma_start(
                                out=out_d[b0:b0 + P, :],
                                in_=osb[:, bj * P:(bj + 1) * P])

            def aggregate(layer, srcA, srcB, post_group=None):
                """Software-pipelined: stage s emits gather-h0(s), mm-h0(s-1),
                gather-h1(s-1), mm-h1(s-2), epilogue(s-2)."""
                gb = group_blocks()
                st = {}   # per-group state: pbf, M0, S0, t00, M1, S1, t01
                for s in range(NGRP + 2):
                    if s < NGRP:
                        gi, blks = gb[s]
                        pbf = [ppool.tile([P, G * P], F32, tag="pbf",
                                          name="pbf") for _ in range(FH)]
                        for bj, j in enumerate(blks):
                            for fh in range(FH):
                                nc.tensor.matmul(
                                    out=pbf[fh][:, bj * P:(bj + 1) * P],
                                    lhsT=asb_local[j][:, fh * P:(fh + 1) * P],
                                    rhs=eyedis[:, j * P:(j + 1) * P],
                                    start=(bj == 0), stop=False)
                        M0, S0, t00 = gathers(gi, 0, srcA, srcB)
                        st[s] = dict(pbf=pbf, M0=M0, S0=S0, t00=t00)
                    if 0 <= s - 1 < NGRP:
                        gi, blks = gb[s - 1]
                        d = st[s - 1]
                        mm_half(gi, 0, d["pbf"], d["M0"], d["S0"], d["t00"],
                                blks)
                        M1, S1, t01 = gathers(gi, 1, srcA, srcB)
                        d.update(M1=M1, S1=S1, t01=t01)
                    if 0 <= s - 2 < NGRP:
                        gi, blks = gb[s - 2]
                        d = st.pop(s - 2)
                        mm_half(gi, 1, d["pbf"], d["M1"], d["S1"], d["t01"],
                                blks)
                        epilogue(layer, gi, d["pbf"], blks)
                        if post_group is not None:
                            post_group(gi)

            # ---------------- layer chain
            # dense0 -> agg0 -> dense1 -> agg1 -> dense2 -> agg2 -> dense3
            # -> agg3.  dense_lw for lw>=1 is emitted inside agg_{lw-1} via
            # post_group (A half after group NGA-1, B half at the end).
            shA1 = dram.tile([NBA * P, HID], BF16, tag="shA", name="shA")
            shB1 = dram.tile([NBB * P, HID], BF16, tag="shB", name="shB")
            fA1 = dram.tile([HALF_A, HID], BF16, tag="fA", name="fA")
            fB1 = dram.tile([HALF_B, HID], BF16, tag="fB", name="fB")
            for j in range(NB):
                dense_block(0, j, shA1, shB1)
                if j == NBA - 1:
                    nc.gpsimd.collective_compute(
                        "AllGather", mybir.AluOpType.bypass,
                        ins=[shA1[:].opt()], outs=[fA1[:].opt()],
                        replica_groups=[list(range(NC))])
                if j == NB - 1:
                    nc.gpsimd.collective_compute(
                        "AllGather", mybir.AluOpType.bypass,
                        ins=[shB1[:].opt()], outs=[fB1[:].opt()],
                        replica_groups=[list(range(NC))])
            srcA, srcB = fA1[:, :], fB1[:, :]

            for lw in (1, 2, 3):
                nextA = dram.tile([NBA * P, HID], BF16, tag="shA", name="shA")
                nextB = dram.tile([NBB * P, HID], BF16, tag="shB", name="shB")
                fullA = dram.tile([HALF_A, HID], BF16, tag="fA", name="fA")
                fullB = dram.tile([HALF_B, HID], BF16, tag="fB", name="fB")

                def post(gi, nextA=nextA, nextB=nextB, fullA=fullA,
                         fullB=fullB, lw=lw):
                    if gi == NGA - 1:
                        for j in range(NBA):
                            dense_block(lw, j, nextA, nextB)
                        nc.gpsimd.collective_compute(
                            "AllGather", mybir.AluOpType.bypass,
                            ins=[nextA[:].opt()], outs=[fullA[:].opt()],
                            replica_groups=[list(range(NC))])
                    if gi == NGRP - 1:
                        for j in range(NBA, NB):
                            dense_block(lw, j, nextA, nextB)
                        nc.gpsimd.collective_compute(
                            "AllGather", mybir.AluOpType.bypass,
                            ins=[nextB[:].opt()], outs=[fullB[:].opt()],
                            replica_groups=[list(range(NC))])

                aggregate(lw - 1, srcA, srcB, post_group=post)
                srcA, srcB = fullA[:, :], fullB[:, :]

            aggregate(3, srcA, srcB)

    nc.compile()
    return nc


# ----------------------------------------------------------------- execution

def _iota_np():
    return np.tile(np.arange(P, dtype=np.float32)[None, :], (P, 1)).astype(
        ml_dtypes.bfloat16)


def _make_in_maps(prep, x, Ws, bs, prelu_a):
    wcat = _pack_wcat(Ws)
    biasp = _pack_bias(bs)
    prelup = np.zeros((P, 2), np.float32)
    prelup[:, 0] = prelu_a[:P]
    prelup[:, 1] = prelu_a[P:]
    iota = _iota_np()
    maps = []
    xf = np.asarray(x, np.float32)
    for c in range(NC):
        maps.append({
            "gidx": prep["gidx"][c],
            "scmp": prep["scmp"][c],
            "sew": prep["sew"][c],
            "iota": iota,
            "eyedis": prep["eyedis"][c],
            "wcat": wcat,
            "bias": biasp,
            "prelua": prelup,
            "disc": prep["disc"][c],
            "xown": _pack_xown(xf, prep["nid"][c]),
        })
    return maps


def _assemble_out(prep, results):
    y = np.zeros((N_NODES, HID), dtype=np.float32)
    nid = prep["nid"]
    for c in range(NC):
        o = results[c]["out"].reshape(FH, NB, P, P)
        for fh in range(FH):
            for j in range(NB):
                nids = nid[c, j]
                ok = nids >= 0
                y[nids[ok], fh * P:(fh + 1) * P] = o[fh, j, :, ok]
    return y


def run(x, edge_index, edge_weight, W1, b1, W2, b2, W3, b3, W4, b4, prelu_a):
    prep = _preprocess(edge_index, edge_weight)
    nc = _build(prep)
    in_maps = _make_in_maps(
        prep, x,
        [np.asarray(W1, np.float32), np.asarray(W2, np.float32),
         np.asarray(W3, np.float32), np.asarray(W4, np.float32)],
        [np.asarray(b1, np.float32), np.asarray(b2, np.float32),
         np.asarray(b3, np.float32), np.asarray(b4, np.float32)],
        np.asarray(prelu_a, np.float32))
    res = run_bass_kernel_spmd(nc, in_maps, core_ids=list(range(NC)))
    return _assemble_out(prep, res.results).astype(np.float32)


def kernel(x, edge_index, edge_weight, W1, b1, W2, b2, W3, b3, W4, b4, prelu_a):
    return run(x, edge_index, edge_weight,
               W1, b1, W2, b2, W3, b3, W4, b4, prelu_a)


# revision 12
# speedup vs baseline: 1.9267x; 1.0405x over previous
"""v4: 4-layer GCN encoder on 8 Trainium2 NeuronCores.

Changes over v3.1:
  - 3-way source-region split (A=28, B1=11, B2=10 blocks per core). Each
    layer runs three AllGathers: AG-A launches ~55% into the previous
    layer's aggregation, AG-B1 at ~77%, AG-B2 at the boundary. The last
    collective is ~5MB, so it lands before the pipeline's part-2 gathers
    need it -- removing the per-layer h1-wait stalls of the 2-way split.
  - Aggregation pipeline: stage s emits [diag+gather-A](s),
    [mmA, gather-B1, gather-B2](s-1), [mmB1, mmB2, epilogue](s-2).
  - 16KB SWDGE scratch (ring = one 1024-idx call, baseline-proven).
"""

import numpy as np
import ml_dtypes

import concourse.bacc as bacc
import concourse.mybir as mybir
import concourse.tile as tile
from concourse.bass_utils import run_bass_kernel_spmd

P = 128
BF16 = mybir.dt.bfloat16
F32 = mybir.dt.float32
I16 = mybir.dt.int16

NC = 8
NB = 49
NPART = 3
RB = [28, 11, 10]                  # blocks per region (A, B1, B2)
RBASE = [0, 28, 39]                # first block of each region
RSZ = [NC * b * P for b in RB]     # region row counts: 28672, 11264, 10240
ROFF = [0, RSZ[0], RSZ[0] + RSZ[1]]
G = 4
NGRP = (NB + G - 1) // G           # 13 groups (last has 1 block)
NPAD = sum(RSZ)                    # 50176
N_NODES = 50000
N_EDGES = 800000
IN_CH = 512
HID = 256
FC_IN = IN_CH // P                 # 4
FH = HID // P                      # 2
CT = 8                             # max tiles per gather call (1024 idx cap)
# post-group hooks: dense+AG for region r fires after this group's epilogue
POST_GRP = [6, 9, 12]              # blocks 0-27 / 28-39 / 40-48 covered


def group_blocks():
    return [(g, list(range(g * G, min((g + 1) * G, NB)))) for g in range(NGRP)]


def _region_of_block(j):
    if j < 28:
        return 0
    return 1 if j < 39 else 2


# ----------------------------------------------------------------- host prep

def _preprocess(edge_index, edge_weight):
    src0 = np.asarray(edge_index[0], dtype=np.int64)
    dst0 = np.asarray(edge_index[1], dtype=np.int64)
    ew0 = np.asarray(edge_weight, dtype=np.float32)
    N = N_NODES

    deg = np.bincount(dst0, weights=ew0.astype(np.float64), minlength=N)
    deg = (deg + 1.0).astype(np.float32)
    dis = (1.0 / np.sqrt(deg)).astype(np.float32)

    indeg = np.bincount(dst0, minlength=N)
    order = np.argsort(-indeg, kind="stable")
    nbins = NC * NB
    rank = np.empty(N, dtype=np.int64)
    rank[order] = np.arange(N)
    bin_of = rank % nbins
    core_of = bin_of % NC
    slot_of = bin_of // NC
    lane_of = rank // nbins
    assert lane_of.max() < P

    ecore = core_of[dst0]
    eslot = slot_of[dst0]
    cnt_cs = np.zeros((NC, NB), dtype=np.int64)
    np.add.at(cnt_cs, (ecore, eslot), 1)

    # rank slots by count desc; deal round-robin into the 3 regions so each
    # region's per-rank profiles match across cores
    slot2blk = np.zeros((NC, NB), dtype=np.int64)
    deal = []
    ri = [0, 0, 0]
    for k in range(NB):
        r = k % NPART
        # keep dealing into regions that still have space
        while ri[r] >= RB[r]:
            r = (r + 1) % NPART
        deal.append((r, ri[r]))
        ri[r] += 1
    for c in range(NC):
        ranked = np.argsort(-cnt_cs[c], kind="stable")
        for k, s in enumerate(ranked):
            r, pos = deal[k]
            slot2blk[c, s] = RBASE[r] + pos
    blk_of = slot2blk[core_of, slot_of]

    reg_of = np.where(blk_of < 28, 0, np.where(blk_of < 39, 1, 2))
    base = np.zeros(N, dtype=np.int64)
    for r in range(NPART):
        m = reg_of == r
        base[m] = ROFF[r] + (core_of[m] * RB[r] + (blk_of[m] - RBASE[r])) * P
    gpos = base + lane_of

    # edge part = src's region
    h_of = reg_of[src0]
    ej = blk_of[dst0]
    ec = core_of[dst0]
    cnt = np.zeros((NC, NB, NPART), dtype=np.int64)
    np.add.at(cnt, (ec, ej, h_of), 1)
    tiles = np.ceil(cnt.max(axis=0) / P).astype(np.int64)
    tiles = np.maximum(tiles, 1)

    tile_base = np.zeros((NB, NPART), dtype=np.int64)
    call_ranges = []                                # (g, part) -> (t0, t1)
    t = 0
    for g, blks in group_blocks():
        for r in range(NPART):
            t0 = t
            for j in blks:
                tile_base[j, r] = t
                t += tiles[j, r]
            call_ranges.append((g, r, t0, t))
    ntiles = int(t)
    nslots = ntiles * P

    ekey = (ec * NB + ej) * NPART + h_of
    esort = np.lexsort((gpos[src0], ekey))
    ks = ekey[esort]
    uniq, inv, counts = np.unique(ks, return_inverse=True, return_counts=True)
    starts = np.zeros_like(counts)
    starts[1:] = np.cumsum(counts)[:-1]
    rib = np.arange(len(esort)) - starts[inv]

    e_c = ec[esort]
    e_j = ej[esort]
    e_h = h_of[esort]
    e_src = src0[esort]
    e_dst = dst0[esort]
    e_w = ew0[esort]
    q = (tile_base[e_j, e_h] + rib // P) * P + rib % P

    gidx = np.zeros((NC, nslots), dtype=np.int16)
    scmp = np.zeros((NC, P, ntiles), dtype=np.float32)
    sew = np.zeros((NC, P, ntiles), dtype=np.float32)

    roff = np.array(ROFF, dtype=np.int64)
    idxv = gpos[e_src] - roff[e_h]
    assert idxv.max() < 32768
    for c in range(NC):
        m = e_c == c
        qc = q[m]
        gidx[c, qc] = idxv[m].astype(np.int16)
        scmp[c, qc % P, qc // P] = lane_of[e_dst[m]].astype(np.float32)
        sew[c, qc % P, qc // P] = e_w[m] * dis[e_dst[m]]

    def wrap(a):
        w = a.reshape(NC, ntiles * 8, 16).transpose(0, 2, 1)
        return np.ascontiguousarray(np.tile(w, (1, 8, 1)))

    disc = np.zeros((NC, P, NB), dtype=np.float32)
    disc[core_of, lane_of, blk_of] = dis

    eyedis = np.zeros((NC, P, NB * P), dtype=np.float32)
    for c in range(NC):
        m = core_of == c
        eyedis[c, lane_of[m], blk_of[m] * P + lane_of[m]] = dis[m]

    nid = np.full((NC, NB, P), -1, dtype=np.int64)
    nid[core_of, blk_of, lane_of] = np.arange(N)

    return dict(
        tiles=tiles, tile_base=tile_base, ntiles=ntiles, nslots=nslots,
        call_ranges=call_ranges,
        gidx=wrap(gidx),
        scmp=scmp.astype(ml_dtypes.bfloat16), sew=sew.astype(ml_dtypes.bfloat16),
        disc=disc, eyedis=eyedis.astype(ml_dtypes.bfloat16),
        dis=dis, gpos=gpos, nid=nid,
    )


def _pack_xown(x, nid_c):
    """x^T tiles for the core's own 49 blocks, one row-block per j:
    xo[j*P + p, fc*P + lane] = x[node(c, j, lane), fc*P + p]."""
    out = np.zeros((NB * P, FC_IN * P), dtype=np.float32)
    for j in range(NB):
        nods = nid_c[j]
        ok = nods >= 0
        xv = x[nods[ok]]
        for fc in range(FC_IN):
            out[j * P:(j + 1) * P, fc * P:fc * P + P][:, ok] = \
                xv[:, fc * P:(fc + 1) * P].T
    return out.astype(ml_dtypes.bfloat16)


def _pack_wcat(Ws):
    cols = []
    for Wl in Ws:
        k = Wl.shape[0]
        for fc in range(k // P):
            cols.append(Wl[fc * P:(fc + 1) * P, :])
    return np.concatenate(cols, axis=1).astype(ml_dtypes.bfloat16)


def _pack_bias(bs):
    out = np.zeros((P, 8), dtype=np.float32)
    for l, b in enumerate(bs):
        for fh in range(FH):
            out[:, l * 2 + fh] = b[fh * P:(fh + 1) * P]
    return out


# ----------------------------------------------------------------- builder

def _build(prep):
    tiles = prep["tiles"]
    tile_base = prep["tile_base"]
    ntiles = prep["ntiles"]
    call_ranges = prep["call_ranges"]
    maxg = [0] * NPART
    for (g, r, t0, t1) in call_ranges:
        maxg[r] = max(maxg[r], t1 - t0)

    nc = bacc.Bacc("TRN2", target_bir_lowering=False, debug=False,
                   num_devices=NC, num_swdge_queues=4,
                   dynamic_dma_scratch_size=16384)
    qctr = [0]

    gidx_d = nc.dram_tensor("gidx", [P, ntiles * 8], I16, kind="ExternalInput")
    scmp_d = nc.dram_tensor("scmp", [P, ntiles], BF16, kind="ExternalInput")
    sew_d = nc.dram_tensor("sew", [P, ntiles], BF16, kind="ExternalInput")
    iota_d = nc.dram_tensor("iota", [P, P], BF16, kind="ExternalInput")
    eyedis_d = nc.dram_tensor("eyedis", [P, NB * P], BF16, kind="ExternalInput")
    wcat_cols = (FC_IN + 3 * FH) * HID
    wcat_d = nc.dram_tensor("wcat", [P, wcat_cols], BF16, kind="ExternalInput")
    bias_d = nc.dram_tensor("bias", [P, 8], F32, kind="ExternalInput")
    prelu_d = nc.dram_tensor("prelua", [P, 2], F32, kind="ExternalInput")
    disc_d = nc.dram_tensor("disc", [P, NB], F32, kind="ExternalInput")
    xown_d = nc.dram_tensor("xown", [NB * P, FC_IN * P], BF16,
                            kind="ExternalInput")
    out_d = nc.dram_tensor("out", [FH * NB * P, P], F32, kind="ExternalOutput")

    w_off = {}
    off = 0
    for l in range(4):
        k = FC_IN if l == 0 else FH
        for fc in range(k):
            w_off[(l, fc)] = off
            off += HID

    with tile.TileContext(nc) as tc:
        with (
            tc.tile_pool(name="res", bufs=1) as res,
            tc.tile_pool(name="xpool", bufs=4) as xpool,
            tc.tile_pool(name="akeep", bufs=1) as akeep,
            tc.tile_pool(name="mpool", bufs=2) as mpool,
            tc.tile_pool(name="spool", bufs=2) as spool,
            tc.tile_pool(name="htpool", bufs=1) as htpool,
            tc.tile_pool(name="opool", bufs=4) as opool,
            tc.tile_pool(name="ppool", bufs=6, space="PSUM") as ppool,
            tc.tile_pool(name="dpsum", bufs=2, space="PSUM") as dpsum,
            tc.tile_pool(name="dram", bufs=2, space="DRAM") as dram,
        ):
            gidx = res.tile([P, ntiles * 8], I16)
            nc.sync.dma_start(out=gidx[:], in_=gidx_d[:])
            scmp = res.tile([P, ntiles], BF16)
            nc.sync.dma_start(out=scmp[:], in_=scmp_d[:])
            sew = res.tile([P, ntiles], BF16)
            nc.sync.dma_start(out=sew[:], in_=sew_d[:])
            iota = res.tile([P, P], BF16)
            nc.sync.dma_start(out=iota[:], in_=iota_d[:])
            eyedis = res.tile([P, NB * P], BF16)
            nc.sync.dma_start(out=eyedis[:], in_=eyedis_d[:])
            wcat = res.tile([P, wcat_cols], BF16)
            nc.sync.dma_start(out=wcat[:], in_=wcat_d[:])
            bias = res.tile([P, 8], F32)
            nc.sync.dma_start(out=bias[:], in_=bias_d[:])
            prelua = res.tile([P, 2], F32)
            nc.sync.dma_start(out=prelua[:], in_=prelu_d[:])
            disc = res.tile([P, NB], F32)
            nc.sync.dma_start(out=disc[:], in_=disc_d[:])

            asb_local = {}
            hT = {}

            def dense_block(layer, j, shards):
                pd_ = dpsum.tile([P, HID], F32, tag="dps", name="pd")
                if layer == 0:
                    xbl = xpool.tile([P, FC_IN * P], BF16, tag="xo", name="xo")
                    nc.sync.dma_start(out=xbl[:],
                                      in_=xown_d[j * P:(j + 1) * P, :])
                    for fc in range(FC_IN):
                        nc.tensor.matmul(
                            out=pd_[:], lhsT=xbl[:, fc * P:(fc + 1) * P],
                            rhs=wcat[:, w_off[(0, fc)]:w_off[(0, fc)] + HID],
                            start=(fc == 0), stop=(fc == FC_IN - 1))
                else:
                    gi, bj = j // G, j % G
                    for fc in range(FH):
                        nc.tensor.matmul(
                            out=pd_[:],
                            lhsT=hT[(gi, fc)][:, bj * P:(bj + 1) * P],
                            rhs=wcat[:, w_off[(layer, fc)]:
                                     w_off[(layer, fc)] + HID],
                            start=(fc == 0), stop=(fc == FH - 1))
                asb = akeep.tile([P, HID], BF16, tag=f"asb{j}", name=f"asb{j}")
                asb_local[j] = asb
                nc.scalar.activation(
                    out=asb[:], in_=pd_[:],
                    func=mybir.ActivationFunctionType.Identity,
                    bias=0.0, scale=disc[:, j:j + 1])
                r = _region_of_block(j)
                b0 = (j - RBASE[r]) * P
                nc.sync.dma_start(out=shards[r][b0:b0 + P, :], in_=asb[:])

            def gathers(gi, r, srcs):
                (g, rr, t0, t1) = call_ranges[gi * NPART + r]
                n = t1 - t0
                M = mpool.tile([P, maxg[r] * HID], BF16, tag=f"M{r}",
                               name=f"M{r}")
                ncalls = (n + CT - 1) // CT
                chunk = (n + ncalls - 1) // ncalls
                for k0 in range(0, n, chunk):
                    k1 = min(k0 + chunk, n)
                    nt_ = k1 - k0
                    nc.gpsimd.dma_gather(
                        out_ap=M[:, k0 * HID:k1 * HID].rearrange(
                            "p (t e) -> p t e", e=HID),
                        in_ap=srcs[r],
                        idxs_ap=gidx[:, (t0 + k0) * 8:(t0 + k1) * 8],
                        num_idxs=nt_ * P,
                        num_idxs_reg=nt_ * P,
                        elem_size=HID,
                        queue_num=qctr[0] % 4,
                    )
                    qctr[0] += 1
                S = spool.tile([P, maxg[r] * P], BF16, tag=f"S{r}",
                               name=f"S{r}")
                s3 = S[:, :n * P].rearrange("p (t e) -> p t e", e=P)
                iob = iota[:].rearrange("p (o e) -> p o e", o=1).broadcast_to(
                    [P, n, P])
                nc.vector.tensor_tensor(
                    out=s3, in0=iob, in1=scmp[:, t0:t1].to_broadcast([P, n, P]),
                    op=mybir.AluOpType.is_equal)
                nc.vector.tensor_tensor(
                    out=s3, in0=s3, in1=sew[:, t0:t1].to_broadcast([P, n, P]),
                    op=mybir.AluOpType.mult)
                return M, S, t0

            def mm_part(gi, r, pbf, M, S, t0, blks):
                for bj, j in enumerate(blks):
                    for t in range(tiles[j, r]):
                        tl = tile_base[j, r] - t0 + t
                        last = (r == NPART - 1 and bj == len(blks) - 1
                                and t == tiles[j, r] - 1)
                        for fh in range(FH):
                            nc.tensor.matmul(
                                out=pbf[fh][:, bj * P:(bj + 1) * P],
                                lhsT=M[:, tl * HID + fh * P:
                                       tl * HID + (fh + 1) * P],
                                rhs=S[:, tl * P:(tl + 1) * P],
                                start=False, stop=last)

            def epilogue(layer, gi, pbf, blks):
                gw = len(blks)
                if layer < 3:
                    for fh in range(FH):
                        ht = htpool.tile([P, G * P], BF16,
                                         tag=f"ht{gi}_{fh}", name=f"ht{gi}_{fh}")
                        nc.scalar.activation(
                            out=ht[:, :gw * P], in_=pbf[fh][:, :gw * P],
                            func=mybir.ActivationFunctionType.Identity,
                            bias=bias[:, layer * 2 + fh:layer * 2 + fh + 1],
                            scale=1.0)
                        hT[(gi, fh)] = ht
                else:
                    for fh in range(FH):
                        osb = opool.tile([P, G * P], F32, tag="osb", name="osb")
                        nc.scalar.activation(
                            out=osb[:, :gw * P], in_=pbf[fh][:, :gw * P],
                            func=mybir.ActivationFunctionType.Prelu,
                            bias=bias[:, 6 + fh:7 + fh],
                            scale=1.0, alpha=prelua[:, fh:fh + 1])
                        for bj, j in enumerate(blks):
                            b0 = (fh * NB + j) * P
                            nc.sync.dma_start(
                                out=out_d[b0:b0 + P, :],
                                in_=osb[:, bj * P:(bj + 1) * P])

            def aggregate(layer, srcs, post_group=None):
                gb = group_blocks()
                st = {}
                for s in range(NGRP + 2):
                    if s < NGRP:
                        gi, blks = gb[s]
                        pbf = [ppool.tile([P, G * P], F32, tag="pbf",
                                          name="pbf") for _ in range(FH)]
                        for bj, j in enumerate(blks):
                            for fh in range(FH):
                                nc.tensor.matmul(
                                    out=pbf[fh][:, bj * P:(bj + 1) * P],
                                    lhsT=asb_local[j][:, fh * P:(fh + 1) * P],
                                    rhs=eyedis[:, j * P:(j + 1) * P],
                                    start=(bj == 0), stop=False)
                        M0, S0, t00 = gathers(gi, 0, srcs)
                        st[s] = dict(pbf=pbf, M0=M0, S0=S0, t00=t00)
                    if 0 <= s - 1 < NGRP:
                        gi, blks = gb[s - 1]
                        d = st[s - 1]
                        mm_part(gi, 0, d["pbf"], d["M0"], d["S0"], d["t00"],
                                blks)
                        M1, S1, t01 = gathers(gi, 1, srcs)
                        M2, S2, t02 = gathers(gi, 2, srcs)
                        d.update(M1=M1, S1=S1, t01=t01, M2=M2, S2=S2, t02=t02)
                    if 0 <= s - 2 < NGRP:
                        gi, blks = gb[s - 2]
                        d = st.pop(s - 2)
                        mm_part(gi, 1, d["pbf"], d["M1"], d["S1"], d["t01"],
                                blks)
                        mm_part(gi, 2, d["pbf"], d["M2"], d["S2"], d["t02"],
                                blks)
                        epilogue(layer, gi, d["pbf"], blks)
                        if post_group is not None:
                            post_group(gi)

            # ---------------- layer chain
            def make_fulls():
                return [dram.tile([RSZ[r], HID], BF16, tag=f"f{r}",
                                  name=f"f{r}", addr_space="Shared")
                        for r in range(NPART)]

            def make_shards():
                return [dram.tile([RB[r] * NC * P // NC, HID], BF16,
                                  tag=f"sh{r}", name=f"sh{r}")
                        for r in range(NPART)]

            def emit_ag(shards, fulls, r):
                nc.gpsimd.collective_compute(
                    "AllGather", mybir.AluOpType.bypass,
                    ins=[shards[r][:].opt()], outs=[fulls[r][:].opt()],
                    replica_groups=[list(range(NC))])

            shards1 = make_shards()
            fulls1 = make_fulls()
            for j in range(NB):
                dense_block(0, j, shards1)
                for r in range(NPART):
                    if j == RBASE[r] + RB[r] - 1:
                        emit_ag(shards1, fulls1, r)
            srcs = [f[:, :] for f in fulls1]

            for lw in (1, 2, 3):
                shards = make_shards()
                fulls = make_fulls()

                def post(gi, shards=shards, fulls=fulls, lw=lw):
                    for r in range(NPART):
                        if gi == POST_GRP[r]:
                            for j in range(RBASE[r], RBASE[r] + RB[r]):
                                dense_block(lw, j, shards)
                            emit_ag(shards, fulls, r)

                aggregate(lw - 1, srcs, post_group=post)
                srcs = [f[:, :] for f in fulls]

            aggregate(3, srcs)

    nc.compile()
    return nc


# ----------------------------------------------------------------- execution

def _iota_np():
    return np.tile(np.arange(P, dtype=np.float32)[None, :], (P, 1)).astype(
        ml_dtypes.bfloat16)


def _make_in_maps(prep, x, Ws, bs, prelu_a):
    wcat = _pack_wcat(Ws)
    biasp = _pack_bias(bs)
    prelup = np.zeros((P, 2), np.float32)
    prelup[:, 0] = prelu_a[:P]
    prelup[:, 1] = prelu_a[P:]
    iota = _iota_np()
    maps = []
    xf = np.asarray(x, np.float32)
    for c in range(NC):
        maps.append({
            "gidx": prep["gidx"][c],
            "scmp": prep["scmp"][c],
            "sew": prep["sew"][c],
            "iota": iota,
            "eyedis": prep["eyedis"][c],
            "wcat": wcat,
            "bias": biasp,
            "prelua": prelup,
            "disc": prep["disc"][c],
            "xown": _pack_xown(xf, prep["nid"][c]),
        })
    return maps


def _assemble_out(prep, results):
    y = np.zeros((N_NODES, HID), dtype=np.float32)
    nid = prep["nid"]
    for c in range(NC):
        o = results[c]["out"].reshape(FH, NB, P, P)
        for fh in range(FH):
            for j in range(NB):
                nids = nid[c, j]
                ok = nids >= 0
                y[nids[ok], fh * P:(fh + 1) * P] = o[fh, j, :, ok]
    return y


def run(x, edge_index, edge_weight, W1, b1, W2, b2, W3, b3, W4, b4, prelu_a):
    prep = _preprocess(edge_index, edge_weight)
    nc = _build(prep)
    in_maps = _make_in_maps(
        prep, x,
        [np.asarray(W1, np.float32), np.asarray(W2, np.float32),
         np.asarray(W3, np.float32), np.asarray(W4, np.float32)],
        [np.asarray(b1, np.float32), np.asarray(b2, np.float32),
         np.asarray(b3, np.float32), np.asarray(b4, np.float32)],
        np.asarray(prelu_a, np.float32))
    res = run_bass_kernel_spmd(nc, in_maps, core_ids=list(range(NC)))
    return _assemble_out(prep, res.results).astype(np.float32)


def kernel(x, edge_index, edge_weight, W1, b1, W2, b2, W3, b3, W4, b4, prelu_a):
    return run(x, edge_index, edge_weight,
               W1, b1, W2, b2, W3, b3, W4, b4, prelu_a)
